# revision 36
# baseline (speedup 1.0000x reference)
"""AGNN (2-layer propagation) Trainium2 Bass kernel, 8-core SPMD.

Sharding: destination-node ranges across 8 cores (12500 nodes/core, padded to
12544 = 98 tiles of 128), per the dst-range graph-parallel strategy. Per core:
  - h0 = relu(x_local @ W1 + b1) via PE (W1 stationary, x^T moving).
  - node records (hn||h, bf16) packed 4 nodes per 256B window -> DRAM table,
    AllGather across cores (window index fits int16 for dma_gather).
  - per-edge phase in ELL layout (dst-node-major, column-major slot stream):
    gpsimd dma_gather fetches source windows; DVE/ACT compute the
    segment softmax (shift-free: logits = beta*cos are bounded) and the
    weighted sums; per-dst reductions via tensor_reduce.
  - classifier + log_softmax on local nodes; host un-permutes and concats.
"""
import sys
import types

sys.path.insert(0, "/opt/trn_rl_repo")

import numpy as np

try:  # optional NTFF profiling hook (enabled by test.py via TRACE)
    import antenv
    if "antenv.axon_hooks" not in sys.modules:
        _hook = [None]
        _m = types.ModuleType("antenv.axon_hooks")
        _m.set_axon_ntff_profile_hook = lambda h: _hook.__setitem__(0, h)
        _m.get_axon_ntff_profile_hook = lambda: _hook[0]
        sys.modules["antenv.axon_hooks"] = _m
        antenv.axon_hooks = _m
        try:
            from trn_agent_boot.trn_boot import _ntff_profile_via_ctypes
            _m.set_axon_ntff_profile_hook(
                _ntff_profile_via_ctypes("/opt/axon/libaxon_pjrt.so"))
        except Exception:
            pass
except Exception:
    pass

import concourse.bass as bass  # noqa: F401
import concourse.mybir as mybir
import concourse.tile as tile
from concourse import bacc
from concourse import library_config
from concourse.bass_utils import run_bass_kernel_spmd
from concourse.masks import make_identity

F32 = mybir.dt.float32
BF16 = mybir.dt.bfloat16
I16 = mybir.dt.int16
AF = mybir.ActivationFunctionType
OP = mybir.AluOpType
AX = mybir.AxisListType

NC_CORES = 8
N = 100000
F_IN = 1433
H = 16
C = 7
EPS = 1e-12

L = N // NC_CORES            # 12500 local nodes per core
LP = 12544                   # padded (98 tiles of 128)
NT = LP // 128               # 98 tiles
KP = 1536                    # padded contraction dim (12 x 128)
NW_CORE = LP // 4            # 3136 windows per core
NW = NC_CORES * NW_CORE      # 25088 real windows
DUMMY_W = NW                 # zero window
NTAB = NW + 4
NIDX_CALL = 1024             # dma_gather safe per-call index count
COLS_CALL = NIDX_CALL // 128  # 8 ELL columns per call
COL_W = NIDX_CALL // 16       # 64 wrapped idx columns per call
GROUP_CAP = 160              # max summed K per gather group (cols)

TRACE = [False]
LAST_EXEC_NS = [None]


def _bc(ap, shape):
    try:
        return ap.broadcast_to(shape)
    except Exception:
        return ap.to_broadcast(shape)


def _plan(deg_sorted):
    """Shared (across cores) tile K profile and gather group/call layout.

    Tiles in a group share a uniform slot width kbar (the group max) so the
    per-group compute can run as single batched DVE ops over [nt, kbar]."""
    Kprof = np.maximum(deg_sorted[:, ::128].max(axis=0), 1).astype(np.int64)
    groups = []          # (tile_list, ncalls, cap, col_base, kbar)
    colbase = np.zeros(NT, dtype=np.int64)
    kbar_of = np.zeros(NT, dtype=np.int64)
    callbase = []
    cols_total = 0
    calls_total = 0
    t = 0
    while t < NT:
        ts = [t]
        sk = int(Kprof[t])
        t += 1
        while t < NT and sk + int(Kprof[t]) <= GROUP_CAP:
            sk += int(Kprof[t])
            ts.append(t)
            t += 1
        ncalls = (sk + COLS_CALL - 1) // COLS_CALL
        cap = ncalls * COLS_CALL
        off = 0
        for tt in ts:
            colbase[tt] = cols_total + off
            kbar_of[tt] = int(Kprof[tt])
            off += int(Kprof[tt])
        callbase.append(calls_total)
        groups.append((ts, ncalls, cap, cols_total, 0))
        cols_total += cap
        calls_total += ncalls
    return Kprof, groups, colbase, kbar_of, callbase, cols_total, calls_total


def _host_prep(x, edge_index, W1, b1, beta, W2, b2):
    deg = np.bincount(edge_index[1], minlength=N)  # self-loops analytic

    perms = []
    deg_sorted = np.zeros((NC_CORES, LP), dtype=np.int64)
    for c in range(NC_CORES):
        dl = deg[c * L:(c + 1) * L]
        order = np.argsort(-dl, kind="stable")
        perms.append(order)
        deg_sorted[c, :L] = dl[order]

    (Kprof, groups, colbase, kbar_of, callbase, cols_total,
     calls_total) = _plan(deg_sorted)
    KMAX = int(Kprof.max())

    gA = 0
    for gi, g in enumerate(groups):
        if g[0][-1] + 1 >= int(0.85 * NT):
            gA = gi + 1
            break
    tA = groups[gA - 1][0][-1] + 1

    src_all = edge_index[0].astype(np.int64)
    dst_all = edge_index[1].astype(np.int64)

    rank_of = np.empty(N, dtype=np.int64)
    for c in range(NC_CORES):
        rank_of[c * L + perms[c]] = np.arange(L)
    src_rank = rank_of[src_all]
    c_src = src_all // L
    w_in_core = src_rank >> 2            # = t*32 + (p>>2), tile-major
    rA = tA * 32
    rB = (NT - tA) * 32
    in_b = w_in_core >= rA
    window = np.where(~in_b, c_src * rA + w_in_core,
                      NC_CORES * rA + c_src * rB + (w_in_core - rA))
    src_gp = window * 4 + (src_rank & 3)
    dstc = dst_all // L
    dst_rank = rank_of[dst_all]

    idx_streams = np.empty((NC_CORES, 128, calls_total * COL_W), dtype=np.int16)
    msks = np.zeros((NC_CORES, 128, 3 * cols_total), dtype=np.uint8)

    for c in range(NC_CORES):
        sel = dstc == c
        dr = dst_rank[sel]
        gp = src_gp[sel]
        o = np.argsort(dr, kind="stable")
        dr = dr[o]; gp = gp[o]
        starts = np.searchsorted(dr, np.arange(LP))
        pos = np.arange(len(dr)) - starts[dr]
        tt = dr // 128
        pp = dr % 128
        col = colbase[tt] + pos
        iw = np.full((128, cols_total), DUMMY_W, dtype=np.int64)
        iw[pp, col] = gp >> 2
        sj = gp & 3
        for j in (1, 2, 3):
            sel = sj == j
            msks[c][pp[sel], (j - 1) * cols_total + col[sel]] = 1
        for gi, (ts, ncalls, cap, colb, kb) in enumerate(groups):
            blk = iw[:, colb:colb + cap]                   # [128, cap]
            lin = blk.T.reshape(ncalls, NIDX_CALL)         # col-major per call
            wr = lin.reshape(ncalls, COL_W, 16).transpose(0, 2, 1)
            flat = wr.transpose(1, 0, 2).reshape(16, ncalls * COL_W)
            cb = callbase[gi]
            for rep in range(8):
                idx_streams[c, rep * 16:(rep + 1) * 16,
                            cb * COL_W:(cb + ncalls) * COL_W] = flat

    padc = np.zeros((NC_CORES, 128, NT), dtype=np.float32)
    for c in range(NC_CORES):
        padc[c] = kbar_of[None, :].astype(np.float32) \
            - deg_sorted[c].reshape(NT, 128).T  # [p, t]

    in_maps = []
    W1p = np.zeros((KP, H), dtype=np.float32)
    W1p[:F_IN] = W1
    for c in range(NC_CORES):
        xt = np.zeros((KP, LP), dtype=np.float32)
        xt[:F_IN, :L] = x[c * L + perms[c]].T
        in_maps.append({
            "xt": xt.astype(np.dtype("bfloat16")),
            "w1": W1p.astype(np.dtype("bfloat16")),
            "b1": b1.reshape(H, 1).astype(np.float32),
            "w2r": np.tile(W2.T.reshape(1, C, H),
                           (128, 1, 1)).reshape(128, C * H)
                .astype(np.float32),
            "b2r": np.tile(b2.reshape(1, C), (128, 1)).astype(np.float32),
            "betar": np.full((128, 1), float(beta[0]), dtype=np.float32),
            "idxs": idx_streams[c],
            "msks": msks[c],
            "padc": padc[c],
        })
    meta = dict(groups=groups, colbase=colbase, callbase=callbase,
                cols_total=cols_total, calls_total=calls_total,
                Kprof=Kprof, KMAX=KMAX, perms=perms, gA=gA, tA=tA)
    return in_maps, meta


def _build_program(meta):
    groups = meta["groups"]
    colbase = meta["colbase"]
    callbase = meta["callbase"]
    cols_total = meta["cols_total"]
    calls_total = meta["calls_total"]
    Kprof = meta["Kprof"]
    KMAX = meta["KMAX"]
    gA = meta["gA"]
    tA = meta["tA"]
    CAPMAX = int(max(g[2] for g in groups))
    NCALLS_MAX = int(max(g[1] for g in groups))

    nc = bacc.Bacc("TRN2", target_bir_lowering=False, debug=False,
                   num_devices=NC_CORES, num_swdge_queues=4)

    xt_d = nc.dram_tensor("xt", [KP, LP], BF16, kind="ExternalInput")
    w1_d = nc.dram_tensor("w1", [KP, H], BF16, kind="ExternalInput")
    b1_d = nc.dram_tensor("b1", [H, 1], F32, kind="ExternalInput")
    w2r_d = nc.dram_tensor("w2r", [128, C * H], F32, kind="ExternalInput")
    b2r_d = nc.dram_tensor("b2r", [128, C], F32, kind="ExternalInput")
    betar_d = nc.dram_tensor("betar", [128, 1], F32, kind="ExternalInput")
    idxs_d = nc.dram_tensor("idxs", [128, calls_total * COL_W], I16,
                            kind="ExternalInput")
    msks_d = nc.dram_tensor("msks", [128, 3 * cols_total], mybir.dt.uint8,
                            kind="ExternalInput")
    padc_d = nc.dram_tensor("padc", [128, NT], F32, kind="ExternalInput")
    out_d = nc.dram_tensor("out", [LP, C], F32, kind="ExternalOutput")

    rec_d = [nc.dram_tensor(f"rec{i}", [NT * 32, 128], BF16)
         for i in range(2)]
    tab_d = [nc.dram_tensor(f"tab{i}", [NTAB, 128], BF16, addr_space="Shared")
             for i in range(2)]

    with tile.TileContext(nc) as tc:
        with tc.tile_pool(name="const", bufs=1) as cst, \
             tc.tile_pool(name="state", bufs=1) as st, \
             tc.tile_pool(name="work", bufs=2) as wk, \
             tc.tile_pool(name="phse", bufs=1) as ph, \
             tc.tile_pool(name="gath", bufs=3) as gp:

            nc.gpsimd.load_library(library_config.mlp)

            w1sb = cst.tile([128, 12, H], BF16)
            for kt in range(12):
                nc.sync.dma_start(out=w1sb[:, kt, :],
                                  in_=w1_d[kt * 128:(kt + 1) * 128, :])
            b1sb = cst.tile([H, 1], F32)
            nc.sync.dma_start(out=b1sb[:], in_=b1_d[:])
            w2rsb = cst.tile([128, C, H], F32)
            nc.sync.dma_start(
                out=w2rsb[:].rearrange("p c h -> p (c h)"), in_=w2r_d[:])
            b2rsb = cst.tile([128, C], F32)
            nc.sync.dma_start(out=b2rsb[:], in_=b2r_d[:])
            betasb = cst.tile([128, 1], F32)
            nc.sync.dma_start(out=betasb[:], in_=betar_d[:])
            msksb = cst.tile([128, 3 * cols_total], mybir.dt.uint8)
            nc.sync.dma_start(out=msksb[:], in_=msks_d[:])
            padsb = cst.tile([128, NT], F32)
            nc.sync.dma_start(out=padsb[:], in_=padc_d[:])
            ident128 = cst.tile([128, 128], F32)
            make_identity(nc, ident128[:])
            zer = cst.tile([1, 128], BF16)
            nc.vector.memset(zer[:], 0)
            for i in range(2):
                nc.sync.dma_start(out=tab_d[i][NW:NW + 1, :], in_=zer[:])

            recS = st.tile([128, NT, 32], BF16)
            h1nm = st.tile([128, NT, H], F32)
            sgrp = st.tile([128, NT], F32)

            def normalize_and_share(hsrc, phase, lo, hi, rs=None):
                # records: hn = hsrc/|hsrc|; |h| = |hsrc| (rs None) or
                # |hsrc|*rs (hsrc is the un-divided numerator, h = hsrc*rs).
                w = hi - lo
                hh = ph.tile([128, NT * H], F32, tag="hh")
                nc.scalar.activation(
                    hh[:, lo * H:hi * H],
                    hsrc[:, lo:hi, :].rearrange("p t h -> p (t h)"),
                    AF.Square)
                ss = ph.tile([128, NT], F32, tag="ss")
                nc.vector.tensor_reduce(
                    ss[:, lo:hi],
                    hh[:, lo * H:hi * H].rearrange("p (t h) -> p t h", h=H),
                    axis=AX.X, op=OP.add)
                nc.vector.tensor_scalar_add(ss[:, lo:hi], ss[:, lo:hi], EPS)
                sq = ph.tile([128, NT], F32, tag="ss2")
                nc.scalar.activation(sq[:, lo:hi], ss[:, lo:hi], AF.Sqrt)
                rr = ph.tile([128, NT], F32, tag="rr")
                nc.vector.reciprocal(rr[:, lo:hi], sq[:, lo:hi])
                nc.vector.tensor_tensor(
                    out=recS[:, lo:hi, 0:H], in0=hsrc[:, lo:hi, :],
                    in1=_bc(rr[:, lo:hi].unsqueeze(2), [128, w, H]),
                    op=OP.mult)
                if rs is None:
                    nc.vector.tensor_copy(out=recS[:, lo:hi, H:H + 1],
                                          in_=sq[:, lo:hi].unsqueeze(2))
                else:
                    hnm2 = ph.tile([128, NT], F32, tag="hnm2")
                    nc.vector.tensor_tensor(out=hnm2[:, lo:hi],
                                            in0=sq[:, lo:hi],
                                            in1=rs[:, lo:hi], op=OP.mult)
                    nc.vector.tensor_copy(out=recS[:, lo:hi, H:H + 1],
                                          in_=hnm2[:, lo:hi].unsqueeze(2))
                nc.sync.dma_start(
                    out=rec_d[phase][:].rearrange(
                        "(t a) (b v) -> (a b) t v", a=32, b=4)[:, lo:hi, :],
                    in_=recS[:, lo:hi, :])
                nc.gpsimd.collective_compute(
                    "AllGather", OP.bypass,
                    replica_groups=[list(range(NC_CORES))],
                    ins=[rec_d[phase][lo * 32:hi * 32, :]],
                    outs=[tab_d[phase][NC_CORES * lo * 32:
                                       NC_CORES * hi * 32, :]],
                )


            # ------------- phase A: h0 = relu(x W1 + b1), node-major -------
            h0nm = st.tile([128, NT, H], F32)
            with tc.tile_pool(name="psA", bufs=2, space="PSUM") as psA, \
                 tc.tile_pool(name="psTa", bufs=2, space="PSUM") as psTa, \
                 tc.tile_pool(name="wkA", bufs=2) as wkA:
                CH = 1536
                for coff in range(0, LP, CH):
                    csz = min(CH, LP - coff)
                    ps = psA.tile([H, CH], F32, tag="psa")
                    for kt in range(12):
                        xtile = wkA.tile([128, CH], BF16, tag="xt")
                        deng = (nc.sync, nc.scalar, nc.gpsimd)[kt % 3]
                        deng.dma_start(
                            out=xtile[:, :csz],
                            in_=xt_d[kt * 128:(kt + 1) * 128, coff:coff + csz])
                        for m in range(0, csz, 512):
                            mw = min(512, csz - m)
                            nc.tensor.matmul(ps[:, m:m + mw],
                                             lhsT=w1sb[:, kt, :],
                                             rhs=xtile[:, m:m + mw],
                                             start=(kt == 0), stop=(kt == 11))
                    hfm = wkA.tile([H, CH], F32, tag="hfm")
                    nc.scalar.activation(hfm[:, :csz], ps[:, :csz], AF.Relu,
                                         bias=b1sb[:])
                    for i in range(csz // 128):
                        tg = (coff + i * 128) // 128
                        pt = psTa.tile([128, H], F32, tag="pst")
                        nc.tensor.transpose(
                            out=pt[:], in_=hfm[:, i * 128:(i + 1) * 128],
                            identity=ident128[:H, :H])
                        nc.vector.tensor_copy(out=h0nm[:, tg, :], in_=pt[:])
                    if (coff + csz) // 128 >= tA > coff // 128:
                        normalize_and_share(h0nm, 0, 0, tA)
            normalize_and_share(h0nm, 0, tA, NT)

            def prop(hio, phase, use_beta, share=None, post=None):
                if use_beta:
                    eb = ph.tile([128, 1], F32, tag="eb")
                    nc.scalar.activation(eb[:], betasb[:], AF.Exp)
                    ebs = eb[:]
                else:
                    ebs = 2.718281828459045
                rs = ph.tile([128, NT], F32, tag="rs")

                def fin_half(lo, hi):
                    # padded slots each contributed exp(0)=1; self-loop adds
                    # exp(beta) to the denominator and exp(beta)*h to the sum.
                    nc.vector.scalar_tensor_tensor(
                        out=sgrp[:, lo:hi], in0=sgrp[:, lo:hi], scalar=ebs,
                        in1=padsb[:, lo:hi], op0=OP.add, op1=OP.subtract)
                    nc.vector.scalar_tensor_tensor(
                        out=h1nm[:, lo:hi, :], in0=hio[:, lo:hi, :],
                        scalar=ebs, in1=h1nm[:, lo:hi, :],
                        op0=OP.mult, op1=OP.add)
                    nc.vector.reciprocal(rs[:, lo:hi], sgrp[:, lo:hi])

                for gi, (ts, ncalls, cap, colb, kb) in enumerate(groups):
                    idxsb = wk.tile([128, NCALLS_MAX * COL_W], I16, tag="idx")
                    cb = callbase[gi]
                    nc.sync.dma_start(
                        out=idxsb[:, :ncalls * COL_W],
                        in_=idxs_d[:, cb * COL_W:(cb + ncalls) * COL_W])
                    G4 = gp.tile([128, CAPMAX, 128], BF16, tag="g4")
                    for cc in range(ncalls):
                        nc.gpsimd.dma_gather(
                            out_ap=G4[:, cc * COLS_CALL:(cc + 1) * COLS_CALL, :],
                            in_ap=tab_d[phase][:],
                            idxs_ap=idxsb[:, cc * COL_W:(cc + 1) * COL_W],
                            num_idxs=NIDX_CALL,
                            num_idxs_reg=NIDX_CALL,
                            elem_size=128,
                            queue_num=cc % 4,
                        )
                    off = 0
                    for t in ts:
                        K = int(Kprof[t])
                        Gt = G4[:, off:off + K, :]
                        Gs = wk.tile([128, KMAX, 18], BF16, tag="gs")
                        nc.vector.tensor_copy(out=Gs[:, :K, :],
                                              in_=Gt[:, :, 0:18])
                        for j in (1, 2, 3):
                            mb = (j - 1) * cols_total + colbase[t]
                            nc.vector.copy_predicated(
                                out=Gs[:, :K, :],
                                mask=_bc(msksb[:, mb:mb + K].unsqueeze(2),
                                         [128, K, 18]),
                                data=Gt[:, :, 32 * j:32 * j + 18])
                        prod = wk.tile([128, KMAX, H], BF16, tag="prod")
                        nc.vector.tensor_tensor(
                            out=prod[:, :K, :], in0=Gs[:, :K, 0:H],
                            in1=_bc(recS[:, t:t + 1, 0:H], [128, K, H]),
                            op=OP.mult)
                        cosr = wk.tile([128, KMAX], F32, tag="cosr")
                        nc.vector.tensor_reduce(cosr[:, :K], prod[:, :K, :],
                                                axis=AX.X, op=OP.add)
                        ee = wk.tile([128, KMAX], F32, tag="ee")
                        nc.scalar.activation(
                            ee[:, :K], cosr[:, :K], AF.Exp,
                            scale=betasb[:] if use_beta else 1.0)
                        nc.vector.tensor_reduce(sgrp[:, t:t + 1], ee[:, :K],
                                                axis=AX.X, op=OP.add)
                        em2 = wk.tile([128, KMAX], F32, tag="em2")
                        nc.vector.tensor_tensor(out=em2[:, :K], in0=ee[:, :K],
                                                in1=Gs[:, :K, H], op=OP.mult)
                        wei = wk.tile([128, KMAX, H], BF16, tag="wei")
                        nc.vector.tensor_tensor(
                            out=wei[:, :K, :], in0=Gs[:, :K, 0:H],
                            in1=_bc(em2[:, :K].unsqueeze(2),
                                    [128, K, H]),
                            op=OP.mult)
                        nc.vector.tensor_reduce(
                            h1nm[:, t, :],
                            wei[:, :K, :].rearrange("p k h -> p h k"),
                            axis=AX.X, op=OP.add)
                        off += K
                    if gi == gA - 1:
                        fin_half(0, tA)
                        if share is not None:
                            normalize_and_share(h1nm, share, 0, tA, rs=rs)
                        elif post is not None:
                            post(0, tA, rs)
                fin_half(tA, NT)
                if share is not None:
                    normalize_and_share(h1nm, share, tA, NT, rs=rs)
                elif post is not None:
                    post(tA, NT, rs)
                return rs

            rs0 = prop(h0nm, 0, use_beta=False, share=1)
            nc.vector.tensor_tensor(
                out=h0nm[:], in0=h1nm[:],
                in1=_bc(rs0[:].unsqueeze(2), [128, NT, H]), op=OP.mult)

            # ---- classifier + log_softmax, emitted per half mid-stream ---
            logits = st.tile([128, NT, C], F32)
            tmpcf = ph.tile([128, NT * H], F32, tag="hh")
            tmpc = tmpcf[:].rearrange("p (t h) -> p t h", h=H)
            m7 = ph.tile([128, NT], F32, tag="m7")
            zm = ph.tile([128, NT, C], F32, tag="zm")
            ez = ph.tile([128, NT, C], F32, tag="ez")
            s7 = ph.tile([128, NT], F32, tag="s7")
            l7 = ph.tile([128, NT], F32, tag="l7")

            def cls_half(lo, hi, rsx):
                w = hi - lo
                nc.vector.tensor_tensor(
                    out=h0nm[:, lo:hi, :], in0=h1nm[:, lo:hi, :],
                    in1=_bc(rsx[:, lo:hi].unsqueeze(2), [128, w, H]),
                    op=OP.mult)
                for c in range(C):
                    nc.vector.tensor_tensor(
                        out=tmpc[:, lo:hi, :], in0=h0nm[:, lo:hi, :],
                        in1=_bc(w2rsb[:, c:c + 1, :], [128, w, H]),
                        op=OP.mult)
                    nc.vector.tensor_reduce(logits[:, lo:hi, c],
                                            tmpc[:, lo:hi, :],
                                            axis=AX.X, op=OP.add)
                nc.vector.tensor_tensor(
                    out=logits[:, lo:hi, :], in0=logits[:, lo:hi, :],
                    in1=_bc(b2rsb[:].unsqueeze(1), [128, w, C]), op=OP.add)
                nc.vector.tensor_reduce(m7[:, lo:hi], logits[:, lo:hi, :],
                                        axis=AX.X, op=OP.max)
                nc.vector.tensor_tensor(
                    out=zm[:, lo:hi, :], in0=logits[:, lo:hi, :],
                    in1=_bc(m7[:, lo:hi].unsqueeze(2), [128, w, C]),
                    op=OP.subtract)
                nc.scalar.activation(
                    ez[:, lo:hi, :].rearrange("p t c -> p (t c)"),
                    zm[:, lo:hi, :].rearrange("p t c -> p (t c)"), AF.Exp)
                nc.vector.tensor_reduce(s7[:, lo:hi], ez[:, lo:hi, :],
                                        axis=AX.X, op=OP.add)
                nc.scalar.activation(l7[:, lo:hi], s7[:, lo:hi], AF.Ln)
                nc.vector.tensor_tensor(
                    out=zm[:, lo:hi, :], in0=zm[:, lo:hi, :],
                    in1=_bc(l7[:, lo:hi].unsqueeze(2), [128, w, C]),
                    op=OP.subtract)
                nc.sync.dma_start(
                    out=out_d[:].rearrange("(p t) c -> p t c",
                                           p=128)[:, lo:hi, :],
                    in_=zm[:, lo:hi, :])

            prop(h0nm, 1, use_beta=True, post=cls_half)

    nc.compile()
    return nc


_CACHE = {}


def kernel(x, edge_index, W1, b1, beta, W2, b2):
    x = np.asarray(x, dtype=np.float32)
    edge_index = np.asarray(edge_index)
    in_maps, meta = _host_prep(x, edge_index, np.asarray(W1), np.asarray(b1),
                               np.asarray(beta), np.asarray(W2),
                               np.asarray(b2))
    if "prog" not in _CACHE:
        _CACHE["prog"] = _build_program(meta)
    nc = _CACHE["prog"]
    res = run_bass_kernel_spmd(nc, in_maps, list(range(NC_CORES)),
                               trace=TRACE[0])
    LAST_EXEC_NS[0] = res.exec_time_ns
    out = np.empty((N, C), dtype=np.float32)
    r = np.arange(L)
    for c in range(NC_CORES):
        oc = res.results[c]["out"].reshape(128, NT, C)
        out[c * L + meta["perms"][c]] = oc[r % 128, r // 128]
    return out



# revision 37
# speedup vs baseline: 1.0622x; 1.0622x over previous
"""AGNN (2-layer propagation) Trainium2 Bass kernel, 8-core SPMD.

Sharding: destination-node ranges across 8 cores (12500 nodes/core, padded to
12544 = 98 tiles of 128), per the dst-range graph-parallel strategy. Per core:
  - h0 = relu(x_local @ W1 + b1) via PE (W1 stationary, x^T moving).
  - node records (hn||h, bf16) packed 4 nodes per 256B window -> DRAM table,
    AllGather across cores (window index fits int16 for dma_gather).
  - per-edge phase in ELL layout (dst-node-major, column-major slot stream):
    gpsimd dma_gather fetches source windows; DVE/ACT compute the
    segment softmax (shift-free: logits = beta*cos are bounded) and the
    weighted sums; per-dst reductions via tensor_reduce.
  - classifier + log_softmax on local nodes; host un-permutes and concats.
"""
import sys
import types

sys.path.insert(0, "/opt/trn_rl_repo")

import numpy as np

try:  # optional NTFF profiling hook (enabled by test.py via TRACE)
    import antenv
    if "antenv.axon_hooks" not in sys.modules:
        _hook = [None]
        _m = types.ModuleType("antenv.axon_hooks")
        _m.set_axon_ntff_profile_hook = lambda h: _hook.__setitem__(0, h)
        _m.get_axon_ntff_profile_hook = lambda: _hook[0]
        sys.modules["antenv.axon_hooks"] = _m
        antenv.axon_hooks = _m
        try:
            from trn_agent_boot.trn_boot import _ntff_profile_via_ctypes
            _m.set_axon_ntff_profile_hook(
                _ntff_profile_via_ctypes("/opt/axon/libaxon_pjrt.so"))
        except Exception:
            pass
except Exception:
    pass

import concourse.bass as bass  # noqa: F401
import concourse.mybir as mybir
import concourse.tile as tile
from concourse import bacc
from concourse import library_config
from concourse.bass_utils import run_bass_kernel_spmd
from concourse.masks import make_identity

F32 = mybir.dt.float32
BF16 = mybir.dt.bfloat16
I16 = mybir.dt.int16
AF = mybir.ActivationFunctionType
OP = mybir.AluOpType
AX = mybir.AxisListType

NC_CORES = 8
N = 100000
F_IN = 1433
H = 16
C = 7
EPS = 1e-12

L = N // NC_CORES            # 12500 local nodes per core
LP = 12544                   # padded (98 tiles of 128)
NT = LP // 128               # 98 tiles
KP = 1536                    # padded contraction dim (12 x 128)
NW_CORE = LP // 4            # 3136 windows per core
NW = NC_CORES * NW_CORE      # 25088 real windows
DUMMY_W = NW                 # zero window
NTAB = NW + 4
NIDX_CALL = 1024             # dma_gather safe per-call index count
COLS_CALL = NIDX_CALL // 128  # 8 ELL columns per call
COL_W = NIDX_CALL // 16       # 64 wrapped idx columns per call
GROUP_CAP = 160              # max summed K per gather group (cols)

TRACE = [False]
LAST_EXEC_NS = [None]


def _bc(ap, shape):
    try:
        return ap.broadcast_to(shape)
    except Exception:
        return ap.to_broadcast(shape)


def _plan(deg_sorted):
    """Shared (across cores) tile K profile and gather group/call layout.

    Tiles in a group share a uniform slot width kbar (the group max) so the
    per-group compute can run as single batched DVE ops over [nt, kbar]."""
    Kprof = np.maximum(deg_sorted[:, ::128].max(axis=0), 1).astype(np.int64)
    groups = []          # (tile_list, ncalls, cap, col_base, kbar)
    colbase = np.zeros(NT, dtype=np.int64)
    kbar_of = np.zeros(NT, dtype=np.int64)
    callbase = []
    cols_total = 0
    calls_total = 0
    t = 0
    while t < NT:
        ts = [t]
        sk = int(Kprof[t])
        t += 1
        while t < NT and sk + int(Kprof[t]) <= GROUP_CAP:
            sk += int(Kprof[t])
            ts.append(t)
            t += 1
        ncalls = (sk + COLS_CALL - 1) // COLS_CALL
        cap = ncalls * COLS_CALL
        off = 0
        for tt in ts:
            colbase[tt] = cols_total + off
            kbar_of[tt] = int(Kprof[tt])
            off += int(Kprof[tt])
        callbase.append(calls_total)
        groups.append((ts, ncalls, cap, cols_total, 0))
        cols_total += cap
        calls_total += ncalls
    return Kprof, groups, colbase, kbar_of, callbase, cols_total, calls_total


def _host_prep(x, edge_index, W1, b1, beta, W2, b2):
    deg = np.bincount(edge_index[1], minlength=N)  # self-loops analytic

    perms = []
    deg_sorted = np.zeros((NC_CORES, LP), dtype=np.int64)
    for c in range(NC_CORES):
        dl = deg[c * L:(c + 1) * L]
        order = np.argsort(-dl, kind="stable")
        perms.append(order)
        deg_sorted[c, :L] = dl[order]

    (Kprof, groups, colbase, kbar_of, callbase, cols_total,
     calls_total) = _plan(deg_sorted)
    KMAX = int(Kprof.max())

    gA = 0
    for gi, g in enumerate(groups):
        if g[0][-1] + 1 >= int(0.85 * NT):
            gA = gi + 1
            break
    tA = groups[gA - 1][0][-1] + 1

    src_all = edge_index[0].astype(np.int64)
    dst_all = edge_index[1].astype(np.int64)

    rank_of = np.empty(N, dtype=np.int64)
    for c in range(NC_CORES):
        rank_of[c * L + perms[c]] = np.arange(L)
    src_rank = rank_of[src_all]
    c_src = src_all // L
    w_in_core = src_rank >> 2            # = t*32 + (p>>2), tile-major
    rA = tA * 32
    rB = (NT - tA) * 32
    in_b = w_in_core >= rA
    window = np.where(~in_b, c_src * rA + w_in_core,
                      NC_CORES * rA + c_src * rB + (w_in_core - rA))
    src_gp = window * 4 + (src_rank & 3)
    dstc = dst_all // L
    dst_rank = rank_of[dst_all]

    idx_streams = np.empty((NC_CORES, 128, calls_total * COL_W), dtype=np.int16)
    msks = np.zeros((NC_CORES, 128, 3 * cols_total), dtype=np.uint8)

    for c in range(NC_CORES):
        sel = dstc == c
        dr = dst_rank[sel]
        gp = src_gp[sel]
        o = np.argsort(dr, kind="stable")
        dr = dr[o]; gp = gp[o]
        starts = np.searchsorted(dr, np.arange(LP))
        pos = np.arange(len(dr)) - starts[dr]
        tt = dr // 128
        pp = dr % 128
        col = colbase[tt] + pos
        iw = np.full((128, cols_total), DUMMY_W, dtype=np.int64)
        iw[pp, col] = gp >> 2
        sj = gp & 3
        for j in (1, 2, 3):
            sel = sj == j
            msks[c][pp[sel], (j - 1) * cols_total + col[sel]] = 1
        for gi, (ts, ncalls, cap, colb, kb) in enumerate(groups):
            blk = iw[:, colb:colb + cap]                   # [128, cap]
            lin = blk.T.reshape(ncalls, NIDX_CALL)         # col-major per call
            wr = lin.reshape(ncalls, COL_W, 16).transpose(0, 2, 1)
            flat = wr.transpose(1, 0, 2).reshape(16, ncalls * COL_W)
            cb = callbase[gi]
            for rep in range(8):
                idx_streams[c, rep * 16:(rep + 1) * 16,
                            cb * COL_W:(cb + ncalls) * COL_W] = flat

    padc = np.zeros((NC_CORES, 128, NT), dtype=np.float32)
    for c in range(NC_CORES):
        padc[c] = kbar_of[None, :].astype(np.float32) \
            - deg_sorted[c].reshape(NT, 128).T  # [p, t]

    in_maps = []
    W1p = np.zeros((KP, H), dtype=np.float32)
    W1p[:F_IN] = W1
    for c in range(NC_CORES):
        xt = np.zeros((KP, LP), dtype=np.float32)
        xt[:F_IN, :L] = x[c * L + perms[c]].T
        in_maps.append({
            "xt": xt.astype(np.dtype("bfloat16")),
            "w1": W1p.astype(np.dtype("bfloat16")),
            "b1": b1.reshape(H, 1).astype(np.float32),
            "w2r": np.tile(W2.T.reshape(1, C, H),
                           (128, 1, 1)).reshape(128, C * H)
                .astype(np.float32),
            "b2r": np.tile(b2.reshape(1, C), (128, 1)).astype(np.float32),
            "betar": np.full((128, 1), float(beta[0]), dtype=np.float32),
            "idxs": idx_streams[c],
            "msks": msks[c],
            "padc": padc[c],
        })
    meta = dict(groups=groups, colbase=colbase, callbase=callbase,
                cols_total=cols_total, calls_total=calls_total,
                Kprof=Kprof, KMAX=KMAX, perms=perms, gA=gA, tA=tA)
    return in_maps, meta


def _build_program(meta):
    groups = meta["groups"]
    colbase = meta["colbase"]
    callbase = meta["callbase"]
    cols_total = meta["cols_total"]
    calls_total = meta["calls_total"]
    Kprof = meta["Kprof"]
    KMAX = meta["KMAX"]
    gA = meta["gA"]
    tA = meta["tA"]
    CAPMAX = int(max(g[2] for g in groups))
    NCALLS_MAX = int(max(g[1] for g in groups))

    nc = bacc.Bacc("TRN2", target_bir_lowering=False, debug=False,
                   num_devices=NC_CORES, num_swdge_queues=4)

    xt_d = nc.dram_tensor("xt", [KP, LP], BF16, kind="ExternalInput")
    w1_d = nc.dram_tensor("w1", [KP, H], BF16, kind="ExternalInput")
    b1_d = nc.dram_tensor("b1", [H, 1], F32, kind="ExternalInput")
    w2r_d = nc.dram_tensor("w2r", [128, C * H], F32, kind="ExternalInput")
    b2r_d = nc.dram_tensor("b2r", [128, C], F32, kind="ExternalInput")
    betar_d = nc.dram_tensor("betar", [128, 1], F32, kind="ExternalInput")
    idxs_d = nc.dram_tensor("idxs", [128, calls_total * COL_W], I16,
                            kind="ExternalInput")
    msks_d = nc.dram_tensor("msks", [128, 3 * cols_total], mybir.dt.uint8,
                            kind="ExternalInput")
    padc_d = nc.dram_tensor("padc", [128, NT], F32, kind="ExternalInput")
    out_d = nc.dram_tensor("out", [LP, C], F32, kind="ExternalOutput")

    rec_d = [nc.dram_tensor(f"rec{i}", [NT * 32, 128], BF16)
         for i in range(2)]
    tab_d = [nc.dram_tensor(f"tab{i}", [NTAB, 128], BF16, addr_space="Shared")
             for i in range(2)]

    with tile.TileContext(nc) as tc:
        with tc.tile_pool(name="const", bufs=1) as cst, \
             tc.tile_pool(name="state", bufs=1) as st, \
             tc.tile_pool(name="work", bufs=2) as wk, \
             tc.tile_pool(name="phse", bufs=1) as ph, \
             tc.tile_pool(name="gath", bufs=2) as gp:

            nc.gpsimd.load_library(library_config.mlp)

            w1sb = cst.tile([128, 12, H], BF16)
            for kt in range(12):
                nc.sync.dma_start(out=w1sb[:, kt, :],
                                  in_=w1_d[kt * 128:(kt + 1) * 128, :])
            b1sb = cst.tile([H, 1], F32)
            nc.sync.dma_start(out=b1sb[:], in_=b1_d[:])
            w2rsb = cst.tile([128, C, H], F32)
            nc.sync.dma_start(
                out=w2rsb[:].rearrange("p c h -> p (c h)"), in_=w2r_d[:])
            b2rsb = cst.tile([128, C], F32)
            nc.sync.dma_start(out=b2rsb[:], in_=b2r_d[:])
            betasb = cst.tile([128, 1], F32)
            nc.sync.dma_start(out=betasb[:], in_=betar_d[:])
            msksb = cst.tile([128, 3 * cols_total], mybir.dt.uint8)
            nc.sync.dma_start(out=msksb[:], in_=msks_d[:])
            padsb = cst.tile([128, NT], F32)
            nc.sync.dma_start(out=padsb[:], in_=padc_d[:])
            ident128 = cst.tile([128, 128], F32)
            make_identity(nc, ident128[:])
            zer = cst.tile([1, 128], BF16)
            nc.vector.memset(zer[:], 0)
            for i in range(2):
                nc.sync.dma_start(out=tab_d[i][NW:NW + 1, :], in_=zer[:])

            recS = st.tile([128, NT, 32], BF16)
            h1nm = st.tile([128, NT, H], F32)
            sgrp = st.tile([128, NT], F32)

            def normalize_and_share(hsrc, phase, lo, hi, rs=None):
                # records: hn = hsrc/|hsrc|; |h| = |hsrc| (rs None) or
                # |hsrc|*rs (hsrc is the un-divided numerator, h = hsrc*rs).
                w = hi - lo
                hh = ph.tile([128, NT * H], F32, tag="hh")
                nc.scalar.activation(
                    hh[:, lo * H:hi * H],
                    hsrc[:, lo:hi, :].rearrange("p t h -> p (t h)"),
                    AF.Square)
                ss = ph.tile([128, NT], F32, tag="ss")
                nc.vector.tensor_reduce(
                    ss[:, lo:hi],
                    hh[:, lo * H:hi * H].rearrange("p (t h) -> p t h", h=H),
                    axis=AX.X, op=OP.add)
                nc.vector.tensor_scalar_add(ss[:, lo:hi], ss[:, lo:hi], EPS)
                sq = ph.tile([128, NT], F32, tag="ss2")
                nc.scalar.activation(sq[:, lo:hi], ss[:, lo:hi], AF.Sqrt)
                rr = ph.tile([128, NT], F32, tag="rr")
                nc.vector.reciprocal(rr[:, lo:hi], sq[:, lo:hi])
                nc.vector.tensor_tensor(
                    out=recS[:, lo:hi, 0:H], in0=hsrc[:, lo:hi, :],
                    in1=_bc(rr[:, lo:hi].unsqueeze(2), [128, w, H]),
                    op=OP.mult)
                if rs is None:
                    nc.vector.tensor_copy(out=recS[:, lo:hi, H:H + 1],
                                          in_=sq[:, lo:hi].unsqueeze(2))
                else:
                    hnm2 = ph.tile([128, NT], F32, tag="hnm2")
                    nc.vector.tensor_tensor(out=hnm2[:, lo:hi],
                                            in0=sq[:, lo:hi],
                                            in1=rs[:, lo:hi], op=OP.mult)
                    nc.vector.tensor_copy(out=recS[:, lo:hi, H:H + 1],
                                          in_=hnm2[:, lo:hi].unsqueeze(2))
                nc.sync.dma_start(
                    out=rec_d[phase][:].rearrange(
                        "(t a) (b v) -> (a b) t v", a=32, b=4)[:, lo:hi, :],
                    in_=recS[:, lo:hi, :])
                nc.gpsimd.collective_compute(
                    "AllGather", OP.bypass,
                    replica_groups=[list(range(NC_CORES))],
                    ins=[rec_d[phase][lo * 32:hi * 32, :]],
                    outs=[tab_d[phase][NC_CORES * lo * 32:
                                       NC_CORES * hi * 32, :]],
                )


            # ------------- phase A: h0 = relu(x W1 + b1), node-major -------
            h0nm = st.tile([128, NT, H], F32)
            with tc.tile_pool(name="psA", bufs=2, space="PSUM") as psA, \
                 tc.tile_pool(name="psTa", bufs=2, space="PSUM") as psTa, \
                 tc.tile_pool(name="wkA", bufs=4) as wkA:
                CH = 1536
                for coff in range(0, LP, CH):
                    csz = min(CH, LP - coff)
                    ps = psA.tile([H, CH], F32, tag="psa")
                    for kt in range(12):
                        xtile = wkA.tile([128, CH], BF16, tag="xt")
                        deng = (nc.sync, nc.scalar, nc.gpsimd)[kt % 3]
                        deng.dma_start(
                            out=xtile[:, :csz],
                            in_=xt_d[kt * 128:(kt + 1) * 128, coff:coff + csz])
                        for m in range(0, csz, 512):
                            mw = min(512, csz - m)
                            nc.tensor.matmul(ps[:, m:m + mw],
                                             lhsT=w1sb[:, kt, :],
                                             rhs=xtile[:, m:m + mw],
                                             start=(kt == 0), stop=(kt == 11))
                    hfm = wkA.tile([H, CH], F32, tag="hfm")
                    nc.scalar.activation(hfm[:, :csz], ps[:, :csz], AF.Relu,
                                         bias=b1sb[:])
                    for i in range(csz // 128):
                        tg = (coff + i * 128) // 128
                        pt = psTa.tile([128, H], F32, tag="pst")
                        nc.tensor.transpose(
                            out=pt[:], in_=hfm[:, i * 128:(i + 1) * 128],
                            identity=ident128[:H, :H])
                        nc.vector.tensor_copy(out=h0nm[:, tg, :], in_=pt[:])
                    if (coff + csz) // 128 >= tA > coff // 128:
                        normalize_and_share(h0nm, 0, 0, tA)
            normalize_and_share(h0nm, 0, tA, NT)

            def prop(hio, phase, use_beta, share=None, post=None):
                if use_beta:
                    eb = ph.tile([128, 1], F32, tag="eb")
                    nc.scalar.activation(eb[:], betasb[:], AF.Exp)
                    ebs = eb[:]
                else:
                    ebs = 2.718281828459045
                rs = ph.tile([128, NT], F32, tag="rs")

                def fin_half(lo, hi):
                    # padded slots each contributed exp(0)=1; self-loop adds
                    # exp(beta) to the denominator and exp(beta)*h to the sum.
                    nc.vector.scalar_tensor_tensor(
                        out=sgrp[:, lo:hi], in0=sgrp[:, lo:hi], scalar=ebs,
                        in1=padsb[:, lo:hi], op0=OP.add, op1=OP.subtract)
                    nc.vector.scalar_tensor_tensor(
                        out=h1nm[:, lo:hi, :], in0=hio[:, lo:hi, :],
                        scalar=ebs, in1=h1nm[:, lo:hi, :],
                        op0=OP.mult, op1=OP.add)
                    nc.vector.reciprocal(rs[:, lo:hi], sgrp[:, lo:hi])

                for gi, (ts, ncalls, cap, colb, kb) in enumerate(groups):
                    idxsb = wk.tile([128, NCALLS_MAX * COL_W], I16, tag="idx")
                    cb = callbase[gi]
                    nc.sync.dma_start(
                        out=idxsb[:, :ncalls * COL_W],
                        in_=idxs_d[:, cb * COL_W:(cb + ncalls) * COL_W])
                    G4 = gp.tile([128, CAPMAX, 128], BF16, tag="g4")
                    for cc in range(ncalls):
                        nc.gpsimd.dma_gather(
                            out_ap=G4[:, cc * COLS_CALL:(cc + 1) * COLS_CALL, :],
                            in_ap=tab_d[phase][:],
                            idxs_ap=idxsb[:, cc * COL_W:(cc + 1) * COL_W],
                            num_idxs=NIDX_CALL,
                            num_idxs_reg=NIDX_CALL,
                            elem_size=128,
                            queue_num=cc % 4,
                        )
                    off = 0
                    for t in ts:
                        K = int(Kprof[t])
                        Gt = G4[:, off:off + K, :]
                        Gs = wk.tile([128, KMAX, 18], BF16, tag="gs")
                        nc.vector.tensor_copy(out=Gs[:, :K, :],
                                              in_=Gt[:, :, 0:18])
                        for j in (1, 2, 3):
                            mb = (j - 1) * cols_total + colbase[t]
                            nc.vector.copy_predicated(
                                out=Gs[:, :K, :],
                                mask=_bc(msksb[:, mb:mb + K].unsqueeze(2),
                                         [128, K, 18]),
                                data=Gt[:, :, 32 * j:32 * j + 18])
                        prod = wk.tile([128, KMAX, H], BF16, tag="prod")
                        nc.vector.tensor_tensor(
                            out=prod[:, :K, :], in0=Gs[:, :K, 0:H],
                            in1=_bc(recS[:, t:t + 1, 0:H], [128, K, H]),
                            op=OP.mult)
                        cosr = wk.tile([128, KMAX], F32, tag="cosr")
                        nc.vector.tensor_reduce(cosr[:, :K], prod[:, :K, :],
                                                axis=AX.X, op=OP.add)
                        ee = wk.tile([128, KMAX], F32, tag="ee")
                        nc.scalar.activation(
                            ee[:, :K], cosr[:, :K], AF.Exp,
                            scale=betasb[:] if use_beta else 1.0)
                        nc.vector.tensor_reduce(sgrp[:, t:t + 1], ee[:, :K],
                                                axis=AX.X, op=OP.add)
                        em2 = wk.tile([128, KMAX], F32, tag="em2")
                        nc.vector.tensor_tensor(out=em2[:, :K], in0=ee[:, :K],
                                                in1=Gs[:, :K, H], op=OP.mult)
                        wei = wk.tile([128, KMAX, H], BF16, tag="wei")
                        nc.vector.tensor_tensor(
                            out=wei[:, :K, :], in0=Gs[:, :K, 0:H],
                            in1=_bc(em2[:, :K].unsqueeze(2),
                                    [128, K, H]),
                            op=OP.mult)
                        nc.vector.tensor_reduce(
                            h1nm[:, t, :],
                            wei[:, :K, :].rearrange("p k h -> p h k"),
                            axis=AX.X, op=OP.add)
                        off += K
                    if gi == gA - 1:
                        fin_half(0, tA)
                        if share is not None:
                            normalize_and_share(h1nm, share, 0, tA, rs=rs)
                        elif post is not None:
                            post(0, tA, rs)
                fin_half(tA, NT)
                if share is not None:
                    normalize_and_share(h1nm, share, tA, NT, rs=rs)
                elif post is not None:
                    post(tA, NT, rs)
                return rs

            rs0 = prop(h0nm, 0, use_beta=False, share=1)
            nc.vector.tensor_tensor(
                out=h0nm[:], in0=h1nm[:],
                in1=_bc(rs0[:].unsqueeze(2), [128, NT, H]), op=OP.mult)

            # ---- classifier + log_softmax, emitted per half mid-stream ---
            logits = st.tile([128, NT, C], F32)
            tmpcf = ph.tile([128, NT * H], F32, tag="hh")
            tmpc = tmpcf[:].rearrange("p (t h) -> p t h", h=H)
            m7 = ph.tile([128, NT], F32, tag="m7")
            zm = ph.tile([128, NT, C], F32, tag="zm")
            ez = ph.tile([128, NT, C], F32, tag="ez")
            s7 = ph.tile([128, NT], F32, tag="s7")
            l7 = ph.tile([128, NT], F32, tag="l7")

            def cls_half(lo, hi, rsx):
                w = hi - lo
                nc.vector.tensor_tensor(
                    out=h0nm[:, lo:hi, :], in0=h1nm[:, lo:hi, :],
                    in1=_bc(rsx[:, lo:hi].unsqueeze(2), [128, w, H]),
                    op=OP.mult)
                for c in range(C):
                    nc.vector.tensor_tensor(
                        out=tmpc[:, lo:hi, :], in0=h0nm[:, lo:hi, :],
                        in1=_bc(w2rsb[:, c:c + 1, :], [128, w, H]),
                        op=OP.mult)
                    nc.vector.tensor_reduce(logits[:, lo:hi, c],
                                            tmpc[:, lo:hi, :],
                                            axis=AX.X, op=OP.add)
                nc.vector.tensor_tensor(
                    out=logits[:, lo:hi, :], in0=logits[:, lo:hi, :],
                    in1=_bc(b2rsb[:].unsqueeze(1), [128, w, C]), op=OP.add)
                nc.vector.tensor_reduce(m7[:, lo:hi], logits[:, lo:hi, :],
                                        axis=AX.X, op=OP.max)
                nc.vector.tensor_tensor(
                    out=zm[:, lo:hi, :], in0=logits[:, lo:hi, :],
                    in1=_bc(m7[:, lo:hi].unsqueeze(2), [128, w, C]),
                    op=OP.subtract)
                nc.scalar.activation(
                    ez[:, lo:hi, :].rearrange("p t c -> p (t c)"),
                    zm[:, lo:hi, :].rearrange("p t c -> p (t c)"), AF.Exp)
                nc.vector.tensor_reduce(s7[:, lo:hi], ez[:, lo:hi, :],
                                        axis=AX.X, op=OP.add)
                nc.scalar.activation(l7[:, lo:hi], s7[:, lo:hi], AF.Ln)
                nc.vector.tensor_tensor(
                    out=zm[:, lo:hi, :], in0=zm[:, lo:hi, :],
                    in1=_bc(l7[:, lo:hi].unsqueeze(2), [128, w, C]),
                    op=OP.subtract)
                nc.sync.dma_start(
                    out=out_d[:].rearrange("(p t) c -> p t c",
                                           p=128)[:, lo:hi, :],
                    in_=zm[:, lo:hi, :])

            prop(h0nm, 1, use_beta=True, post=cls_half)

    nc.compile()
    return nc


_CACHE = {}


def kernel(x, edge_index, W1, b1, beta, W2, b2):
    x = np.asarray(x, dtype=np.float32)
    edge_index = np.asarray(edge_index)
    in_maps, meta = _host_prep(x, edge_index, np.asarray(W1), np.asarray(b1),
                               np.asarray(beta), np.asarray(W2),
                               np.asarray(b2))
    if "prog" not in _CACHE:
        _CACHE["prog"] = _build_program(meta)
    nc = _CACHE["prog"]
    res = run_bass_kernel_spmd(nc, in_maps, list(range(NC_CORES)),
                               trace=TRACE[0])
    LAST_EXEC_NS[0] = res.exec_time_ns
    out = np.empty((N, C), dtype=np.float32)
    r = np.arange(L)
    for c in range(NC_CORES):
        oc = res.results[c]["out"].reshape(128, NT, C)
        out[c * L + meta["perms"][c]] = oc[r % 128, r // 128]
    return out

